# revision 86
# baseline (speedup 1.0000x reference)
"""AdvisorCrossAttentionAdapter Trainium2 kernel (v5).

Full inputs in, full outputs out. Sharding: 8 cores = 4 batches x 2 "halves".
Core 2b+j handles batch b; j indexes both its 1024-row query slice and its
512-triplet share of the per-batch K/V prep. The pair exchanges prep results
(2MB each way) with cheap ReduceScatter collectives; attention runs fully
local after that.

Math notes (all folds are exact; host precomputes products in fp32):
  - K-side fold: scores = hidden @ M @ a0.T with M = Wq.T @ Wk, computed as
    kM = a0 @ M.T on device (T=1024 rows, half the flops of the q-side fold
    used previously, which projected S=2048 rows).
  - Wo folded through the value path:
      vo   = adv_lin @ Wvo + Pc @ (|sc @ Wv.T| @ Wo.T),  Wvo = Wv.T @ Wo.T
      out  = softmax(scores) @ vo
    This deletes the separate ctx (attn @ v_final) + out-projection phases.
    The id-gate decomposition (adv_lin linear part, sc compact abs rows, Pc
    signed scatter) is unchanged from v2 but built per t-half.
  - Prep is row-split (by T) across the pair with LOCAL t ordering
    [own 512 | peer 512] on both kM and vo; softmax/ctx are permutation
    invariant in t so no global order is needed.
  - Pair exchange via ReduceScatter(add): each core writes its own half into
    the peer-destined shard and zeros into its own shard (two ACT copies
    scaled by a host-fed 0/1 mask keep the program SPMD-symmetric). RS output
    is half the bytes of an AllGather output, which halves the collective
    cost under the serialized COLLECTIVE_CORES device.
  - Softmax runs without max subtraction (scores/sqrt(h) ~ N(0,1)); exp'd
    scores stay unnormalized through the out matmul and the 1/sum factor is
    applied on the final ACT copy (per-partition scale).
  - All matmuls take bf16 inputs with fp32 PSUM accumulation.
"""

import numpy as np
import ml_dtypes
from contextlib import ExitStack

P = 128
H = 2048          # hidden dim
HC = H // P       # 16 h-chunks of 128
T = 1024          # triplets per batch (advisor len 3072 / 3)
TC = T // P       # 8 t-chunks
TH = T // 2       # own t-half rows (512)
THC = TH // P     # 4 own t-chunks
S = 1024          # query rows per core (2048 / 2)
B = 4
NCORES = 8
CHC_MIN = 256     # compact rows per t-half, padded to 128 (data-keyed)
SCALE = 1.0 / float(np.sqrt(H))

bf16 = ml_dtypes.bfloat16

_compiled_nc = None


def _build_nc5(s_rows=S, t_trip=T, h=H, chc=CHC_MIN, n_dev=NCORES,
               debug=False):
    import concourse.mybir as mybir
    import concourse.tile as tile
    from concourse import bacc

    hc = h // P
    tc_n = t_trip // P
    th = t_trip // 2
    thc = th // P
    sc_n = s_rows // P
    s512 = s_rows // 512
    n512 = h // 512
    chb = chc // P
    assert s_rows % 512 == 0 and h % 1024 == 0 and chc % P == 0

    f32 = mybir.dt.float32
    bf = mybir.dt.bfloat16
    f8 = mybir.dt.float8e4

    nc = bacc.Bacc("TRN2", target_bir_lowering=False, debug=False,
                   num_devices=n_dev)

    # DRAM I/O (all bf16, host pre-transposed):
    #   mt  [hc, P, h]  : chunked M.T  (o-chunk major; lhsT tiles [i, o])
    #   a0t [P, hc, th] : a0.T own t-half columns (rhs for kM)
    #   wv  [hc, P, h]  : chunked Wv.T (j-chunk major; lhsT tiles [i, j])
    #   sct [P, hc, chc]: sc.T own compact rows (rhs for absT)
    #   wo  [P, hc, h]  : Wo.T  (rhs stream for acWo)
    #   wvo [P, hc, h]  : (Wv.T @ Wo.T)  (rhs stream for vo_lin)
    #   alt [P, hc, th] : adv_lin.T own columns (lhsT for vo_lin)
    #   pct [P, chb, th]: Pc.T own (lhsT for scatter)
    #   hT  [P, hc, s]  : hidden.T own query slice (rhs for scores)
    #   msk [P, 2] f32  : RS shard masks (1.0 on peer-destined shard)
    d_mt = nc.dram_tensor("mt", [hc, 2, P, h], f8, kind="ExternalInput")
    d_a0t = nc.dram_tensor("a0t", [P, 2, hc, th], f8, kind="ExternalInput")
    d_wv = nc.dram_tensor("wv", [hc, P, h], f8, kind="ExternalInput")
    d_sct = nc.dram_tensor("sct", [P, 2, hc, chc], f8, kind="ExternalInput")
    d_wo = nc.dram_tensor("wo", [P, hc, h], bf, kind="ExternalInput")
    d_wvo = nc.dram_tensor("wvo", [P, hc, h], bf, kind="ExternalInput")
    d_alt = nc.dram_tensor("alt", [P, hc, th], bf, kind="ExternalInput")
    d_pct = nc.dram_tensor("pct", [P, chb, th], bf, kind="ExternalInput")
    d_h = nc.dram_tensor("hT", [P, hc, s_rows], bf, kind="ExternalInput")
    d_msk = nc.dram_tensor("msk", [P, 2], f32, kind="ExternalInput")
    d_out = nc.dram_tensor("out", [s_rows, h], f32, kind="ExternalOutput")
    if debug:
        d_dbg_km = nc.dram_tensor("dbg_km", [h, t_trip], f32,
                                  kind="ExternalOutput")
        d_dbg_vo = nc.dram_tensor("dbg_vo", [t_trip, h], f32,
                                  kind="ExternalOutput")
        d_dbg_e = nc.dram_tensor("dbg_e", [t_trip, s_rows], f32,
                                 kind="ExternalOutput")
        d_dbg_rc = nc.dram_tensor("dbg_rc", [P, 8], f32,
                                  kind="ExternalOutput")

    AF = mybir.ActivationFunctionType
    pairs = [[2 * i, 2 * i + 1] for i in range(n_dev // 2)]

    with tile.TileContext(nc) as tc, ExitStack() as ctx:
        big = ctx.enter_context(tc.tile_pool(name="big", bufs=1))
        pws = ctx.enter_context(tc.tile_pool(name="pws", bufs=6))
        pw = ctx.enter_context(tc.tile_pool(name="pw", bufs=3))
        pgs = ctx.enter_context(tc.tile_pool(name="pgs", bufs=4))
        pgo = ctx.enter_context(tc.tile_pool(name="pgo", bufs=3))
        psm = ctx.enter_context(tc.tile_pool(name="psm", bufs=1))
        pp = ctx.enter_context(tc.tile_pool(name="pp", bufs=8, space="PSUM"))
        dram = ctx.enter_context(tc.tile_pool(name="dram", bufs=1,
                                              space="DRAM"))

        # Persistent SBUF intermediates. Tag chains reuse slots across phases:
        #   tag A: a0t (P1)  -> hT (P3)      [32KB/part]
        #   tag L: alt (P2)  -> eT (P3/P4)   [16KB/part]
        kmT = big.tile([P, hc, t_trip], bf, tag="K", name="kmT")
        vo = big.tile([P, tc_n, h], bf, tag="V", name="vo")
        a0t_sb = big.tile([P, 2, hc, th], f8, tag="A", name="a0t_sb")
        alt_sb = big.tile([P, hc, th], bf, tag="L", name="alt_sb")
        sct_sb = big.tile([P, 2, hc, chc], f8, tag="S", name="sct_sb")
        absT_sb = big.tile([P, hc, chc], bf, tag="B", name="absT_sb")
        acWo_sb = big.tile([P, chb, h], bf, tag="W", name="acWo_sb")
        pct_sb = psm.tile([P, chb, th], bf, tag="pc", name="pct_sb")
        msk_sb = psm.tile([P, 2], f32, tag="mk", name="msk_sb")

        # Exchange buffers (internal DRAM). km_in shard s holds this core's
        # kM.T half scaled by msk[s]; RS(add) then delivers the peer's half.
        km_in = dram.tile([2, h, th], bf, name="km_in", uniquify=False)
        km_out = dram.tile([h, th], bf, name="km_out", uniquify=False)
        # Permuted views so ONE staging DMA per PSUM tile writes both shards.
        km_rv = km_in.rearrange("s (oc p) t -> p oc s t", p=P)
        vo_in = []
        vo_out = []
        vo_rv = []
        for k in range(2):
            vo_in.append(dram.tile([2, th, h // 2], bf, name=f"vo_in{k}",
                                   uniquify=False))
            vo_out.append(dram.tile([th, h // 2], bf, name=f"vo_out{k}",
                                    uniquify=False))
            vo_rv.append(vo_in[k].rearrange("s (tb p) o -> p tb s o", p=P))

        # First critical input DMAs. a0t quarter-split (alternating queues)
        # so P1's first matmul group can start before the whole 2MB lands.
        # Queue discipline: sync carries ONLY pure input streams (never a DMA
        # that waits on local compute), so it can never head-of-line block;
        # compute-dependent staging writes all go on scalar.
        qs = max(1, hc // 4)
        nc.scalar.dma_start(a0t_sb[:, 0, 0:hc // 2, :],
                            d_a0t[:, 0, 0:hc // 2, :])
        nc.scalar.dma_start(a0t_sb[:, 0, hc // 2:hc, :],
                            d_a0t[:, 0, hc // 2:hc, :])
        nc.gpsimd.dma_start(msk_sb[:], d_msk[:])
        nc.gpsimd.dma_start(pct_sb[:], d_pct[:])

        # ACT-written zero bias vector (x*0) so Abs/Exp activations don't pull
        # in a DMA'd const AP; also absorbs the pipeline-RAW wait.
        zbias = psm.tile([P, 1], f32, tag="zb", name="zbias")
        nc.scalar.mul(zbias[:], msk_sb[:, 0:1], 0.0)
        warm = psm.tile([P, 1], f32, tag="wm", name="warm")
        nc.scalar.copy(warm[:], zbias[:])

        # PE warm-up: throwaway matmuls while the first weight tiles land so
        # the PE p-state ramp completes before real groups start.
        dummy = psm.tile([P, 512], bf, tag="dm", name="dummy")
        nc.vector.memset(dummy[:], 0.0)
        for _ in range(16):
            ps_dm = pp.tile([P, 512], f32, tag="PS", name="ps_dm")
            nc.tensor.matmul(ps_dm[:], dummy[:, 0:P], dummy[:],
                             start=True, stop=True)

        # ------------- P1: kM.T own columns = M @ a0_own.T ------------------
        # kmT local t order: [own 512 | peer 512]. The mt/wv streams alternate
        # sync/scalar and are issued AHEAD of the compute-dependent staging
        # writes in FIFO order (wv prefetches are hoisted into the P1 loop) so
        # the queues never head-of-line block the weight streams.
        npre = 5
        stream_tiles = {}

        def q_of(i):
            # Streams ride the scalar (ACT) HWDGE queue; compute-dependent
            # writes ride sync (SP) so their desc-gen never clogs ACT's SEQ
            # and streams never sit behind compute-gated writes.
            return nc.scalar

        for oc in range(npre):
            mt_oc = pws.tile([P, 2, hc, P], f8, tag="W1", name="mt_oc")
            stream_tiles[oc] = mt_oc
            if oc == 0:
                nc.scalar.dma_start(mt_oc[:, 0], d_mt[oc, 0])
                nc.scalar.dma_start(mt_oc[:, 1], d_mt[oc, 1])
                # a0t lo-term lands while group 0's hi terms run.
                nc.scalar.dma_start(a0t_sb[:, 1, 0:hc // 2, :],
                                    d_a0t[:, 1, 0:hc // 2, :])
                nc.scalar.dma_start(a0t_sb[:, 1, hc // 2:hc, :],
                                    d_a0t[:, 1, hc // 2:hc, :])
            else:
                q_of(oc).dma_start(mt_oc[:, 0], d_mt[oc, 0])
                q_of(oc + 1).dma_start(mt_oc[:, 1], d_mt[oc, 1])
        wtiles = {}
        for oc in range(hc):
            nx = oc + npre
            if nx < hc:
                t_nx = pws.tile([P, 2, hc, P], f8, tag="W1", name="mt_oc")
                stream_tiles[nx] = t_nx
                q_of(nx).dma_start(t_nx[:, 0], d_mt[nx, 0])
                q_of(nx + 1).dma_start(t_nx[:, 1], d_mt[nx, 1])
            else:
                # wv (fp8) comes in PAIRS: two 2KB tiles share one 4KB slot,
                # doubling the prefetch depth for the DMA-bound absT phase.
                pq2 = nx - hc
                t_nx = pws.tile([P, 2, hc, P], f8, tag="W1", name="wv_jc")
                stream_tiles[hc + pq2] = t_nx
                q_of(nx).dma_start(t_nx[:, 0], d_wv[2 * pq2])
                q_of(nx + 1).dma_start(t_nx[:, 1], d_wv[2 * pq2 + 1])
            if oc == 6:
                # sct (hi/lo fp8 terms) lands mid-P1 (needed from P2 on).
                nc.sync.dma_start(sct_sb[:, 0, :, :], d_sct[:, 0, :, :])
                nc.scalar.dma_start(sct_sb[:, 1, :, :], d_sct[:, 1, :, :])
            if oc in (12, 14):
                # Prefetch the first two wo tiles during P1's tail.
                wi = (oc - 12) // 2
                wt = pw.tile([P, hc, 256], bf, tag="W4", name="wo_ot")
                wtiles[("wo", wi)] = wt
                q_of(oc).dma_start(wt[:],
                                   d_wo[:, :, wi * 256:(wi + 1) * 256])
            mt_oc = stream_tiles.pop(oc)
            # 16x-scaled kM via 3-term fp8 DoubleRow (hi.hi + hi.lo + lo.hi;
            # both splits are exact host-side, so accuracy ~bf16). Two
            # 256-wide halves in SEPARATE banks (start resets a whole bank).
            pshs = []
            for hf in range(2):
                hsl = slice(hf * 256, (hf + 1) * 256)
                ps_k = pp.tile([P, 512], f32, tag="PS", name="ps_k")
                pshs.append(ps_k)
                for tn, (ta, tb2) in enumerate(((0, 0), (0, 1), (1, 0))):
                    for ii in range(hc // 2):
                        nc.tensor.matmul(
                            ps_k[:, 0:256],
                            mt_oc[:, ta, 2 * ii:2 * ii + 2, :],
                            a0t_sb[:, tb2, 2 * ii:2 * ii + 2, hsl],
                            start=(tn == 0 and ii == 0),
                            stop=(tn == 2 and ii == hc // 2 - 1),
                            perf_mode=mybir.MatmulPerfMode.DoubleRow)
                nc.vector.tensor_copy(kmT[:, oc, hsl], ps_k[:, 0:256])
            stg = pgs.tile([P, 2, th], bf, tag="VS", name="stg_k")
            for sh in range(2):
                for hf in range(2):
                    nc.scalar.activation(
                        stg[:, sh, hf * 256:(hf + 1) * 256],
                        pshs[hf][:, 0:256], AF.Copy,
                        scale=msk_sb[:, sh:sh + 1])
            nc.sync.dma_start(km_rv[:, oc], stg[:])
        nc.gpsimd.collective_compute(
            "ReduceScatter",
            mybir.AluOpType.add,
            replica_groups=pairs,
            ins=[km_in.opt()],
            outs=[km_out.opt()],
        )
        # The kM scatter-back is emitted AFTER the first vo RS (see below):
        # emitted here it would hold the gpsimd SEQ through its wait on the
        # kM RS, delaying the vo RS dispatches by ~20us.

        # ------------- P2: value path -> vo (local t order) -----------------
        # absT[j, c] = |sc @ Wv.T|.T for own compact rows.
        npair = hc // 2
        for q in range(npair):
            nxp = q + npre
            if nxp < npair:
                t_nx = pws.tile([P, 2, hc, P], f8, tag="W1", name="wv_jc")
                stream_tiles[hc + nxp] = t_nx
                q_of(nxp).dma_start(t_nx[:, 0], d_wv[2 * nxp])
                q_of(nxp + 1).dma_start(t_nx[:, 1], d_wv[2 * nxp + 1])
            wv_pair = stream_tiles.pop(hc + q)
            for half in range(2):
                jc = 2 * q + half
                ps_a = pp.tile([P, 512], f32, tag="PS", name="ps_a")
                for term in range(2):
                    for ii in range(hc // 2):
                        nc.tensor.matmul(
                            ps_a[:, 0:chc],
                            wv_pair[:, half, 2 * ii:2 * ii + 2, :],
                            sct_sb[:, term, 2 * ii:2 * ii + 2, :],
                            start=(term == 0 and ii == 0),
                            stop=(term == 1 and ii == hc // 2 - 1),
                            perf_mode=mybir.MatmulPerfMode.DoubleRow)
                nc.scalar.activation(absT_sb[:, jc, :], ps_a[:, 0:chc],
                                     AF.Abs, bias=zbias[:])

        # hT for P3 (reuses a0t's slot via tag A chain; a0t is dead after
        # P1). Loaded in 512KB pieces spread through the vo loop and P3-st0
        # so no single transfer hogs the serial DMA pool during P2's crunch.
        hT = big.tile([P, hc, s_rows], bf, tag="A", name="hT")

        # acWo[c, o] = absT.T @ Wo.T over all o, per half-o-tile (256 cols).
        # The wo/wvo streams keep a 2-deep lookahead on alternating queues;
        # alt and hT-half0 pieces ride this window's DMA slack.
        ho = 256
        nho = h // ho
        per_k = nho // 2
        issued = set(wtiles.keys())

        def fetch_w(key):
            if key in issued:
                return
            issued.add(key)
            kind, i = key
            wt = pw.tile([P, hc, ho], bf, tag="W4", name=f"{kind}_ot")
            wtiles[key] = wt
            src = d_wo if kind == "wo" else d_wvo
            q_of(i + (0 if kind == "wo" else 1)).dma_start(
                wt[:], src[:, :, i * ho:(i + 1) * ho])

        for ot in range(nho):
            osl = slice(ot * ho, (ot + 1) * ho)
            for nx in (ot + 1, ot + 2):
                fetch_w(("wo", nx) if nx < nho else ("wvo", nx - nho))
            if ot < 2:
                hq = hc // 2
                q_of(ot).dma_start(alt_sb[:, ot * hq:(ot + 1) * hq, :],
                                   d_alt[:, ot * hq:(ot + 1) * hq, :])
            if ot >= nho - 4:
                # First s-half of hT, in 512KB pieces, on sync (idle here).
                pq = (ot - (nho - 4)) * qs
                nc.sync.dma_start(hT[:, pq:pq + qs, 0:512],
                                  d_h[:, pq:pq + qs, 0:512])
            wo_ot = wtiles.pop(("wo", ot))
            for cb in range(chb):
                ps_c = pp.tile([P, 512], f32, tag="PS", name="ps_c")
                for jc in range(hc):
                    nc.tensor.matmul(ps_c[:, 0:ho],
                                     absT_sb[:, jc, cb * P:(cb + 1) * P],
                                     wo_ot[:, jc, :],
                                     start=(jc == 0), stop=(jc == hc - 1))
                nc.vector.tensor_copy(acWo_sb[:, cb, osl], ps_c[:, 0:ho])
        # vo own rows = adv_lin @ Wvo + Pc @ acWo, per half-o-tile; each
        # 1024-col chunk is exchanged as soon as its staging lands.
        for ot in range(nho):
            osl = slice(ot * ho, (ot + 1) * ho)
            for nx in (ot + 1, ot + 2):
                if nx < nho:
                    fetch_w(("wvo", nx))
            wvo_ot = wtiles.pop(("wvo", ot))
            k, col = ot // per_k, (ot % per_k) * ho
            for tb in range(thc):
                ps_v = pp.tile([P, 512], f32, tag="PS", name="ps_v")
                for ic in range(hc):
                    nc.tensor.matmul(ps_v[:, 0:ho],
                                     alt_sb[:, ic, tb * P:(tb + 1) * P],
                                     wvo_ot[:, ic, :],
                                     start=(ic == 0), stop=False)
                for cb in range(chb):
                    nc.tensor.matmul(ps_v[:, 0:ho],
                                     pct_sb[:, cb, tb * P:(tb + 1) * P],
                                     acWo_sb[:, cb, osl],
                                     start=False, stop=(cb == chb - 1))
                nc.vector.tensor_copy(vo[:, tb, osl], ps_v[:, 0:ho])
                stg = pgs.tile([P, 2, ho], bf, tag="VS2", name="stg_v")
                for sh in range(2):
                    nc.scalar.activation(stg[:, sh, :], ps_v[:, 0:ho],
                                         AF.Copy,
                                         scale=msk_sb[:, sh:sh + 1])
                nc.sync.dma_start(vo_rv[k][:, tb, :, col:col + ho], stg[:])
            if (ot + 1) % per_k == 0:
                nc.gpsimd.collective_compute(
                    "ReduceScatter",
                    mybir.AluOpType.add,
                    replica_groups=pairs,
                    ins=[vo_in[k].opt()],
                    outs=[vo_out[k].opt()],
                )
                if k == 0:
                    # Peer kM columns land via SWDGE. Emitted here (after the
                    # vo chunk-0 RS dispatch) its RS1-sem wait is already
                    # satisfied, so it never blocks the gpsimd SEQ.
                    nc.gpsimd.dma_start(
                        kmT[:, :, th:t_trip],
                        km_out.rearrange("(oc p) t -> p oc t", p=P))
                nc.gpsimd.dma_start(
                    vo[:, thc:tc_n, k * (h // 2):(k + 1) * (h // 2)],
                    vo_out[k].rearrange("(tb p) o -> p tb o", p=P))

        # ------------- P3: scores^T, exp, sums ------------------------------
        # st-major so the first s-half only needs the first half of hT. Row
        # sums accumulate directly into per-s-chunk PSUM columns ([128,1]
        # matmuls against a ones column), giving per-partition reciprocals
        # with no transpose step; emission is deferred one t-chunk so the PE
        # never waits on the ACT exp.
        eT = big.tile([P, tc_n, s_rows], bf, tag="L", name="eT")
        ones_t = psm.tile([P, 1], bf, tag="o1", name="ones_t")
        nc.vector.memset(ones_t[:], 1.0)
        rcol = psm.tile([P, sc_n], f32, tag="rl", name="rcol")

        def emit_sums(st):
            # Row sums as [128,1] accumulation groups, one PSUM TILE each
            # (start resets the whole addressed bank row per partition, so
            # distinct groups must use distinct banks). Output is already
            # [s-part, 1]: the reciprocal lands straight in rcol.
            for sc2 in range(4):
                sc = st * 4 + sc2
                ps_s = pp.tile([P, 512], f32, tag="PS", name="ps_s")
                for tch in range(tc_n):
                    nc.tensor.matmul(ps_s[:, 0:1],
                                     eT[:, tch, sc * P:(sc + 1) * P],
                                     ones_t[:],
                                     start=(tch == 0), stop=(tch == tc_n - 1))
                nc.vector.reciprocal(rcol[:, sc:sc + 1], ps_s[:, 0:1])

        for st in range(s512):
            for tch in range(tc_n):
                if st == 0 and tch >= tc_n - 4:
                    # Second s-half of hT lands piecewise during st=0.
                    pq = (tch - (tc_n - 4)) * qs
                    q_of(tch).dma_start(hT[:, pq:pq + qs, 512:s_rows],
                                        d_h[:, pq:pq + qs, 512:s_rows])
                ps_x = pp.tile([P, 512], f32, tag="PS", name="ps_sc")
                for oc in range(hc):
                    nc.tensor.matmul(ps_x[:],
                                     kmT[:, oc, tch * P:(tch + 1) * P],
                                     hT[:, oc, st * 512:(st + 1) * 512],
                                     start=(oc == 0), stop=(oc == hc - 1))
                if st > 0 and tch == 1:
                    emit_sums(st - 1)
                nc.scalar.activation(eT[:, tch, st * 512:(st + 1) * 512],
                                     ps_x[:], AF.Exp, bias=zbias[:],
                                     scale=SCALE / 16.0)
            if st == s512 - 1:
                emit_sums(st)
        warm2 = psm.tile([P, 1], f32, tag="w2", name="warm2")
        nc.scalar.copy(warm2[:], rcol[:, 0:1])

        if debug:
            def dump(dst, rows, width, src_of):
                for r in range(rows):
                    for w in range(0, width, 512):
                        dd = pgo.tile([P, 512], f32, tag="OB", name="ob")
                        nc.vector.tensor_copy(dd[:], src_of(r, w))
                        nc.sync.dma_start(
                            dst[r * P:(r + 1) * P, w:w + 512], dd[:])
            dump(d_dbg_km, hc, t_trip, lambda r, w: kmT[:, r, w:w + 512])
            dump(d_dbg_vo, tc_n, h, lambda r, w: vo[:, r, w:w + 512])
            dump(d_dbg_e, tc_n, s_rows, lambda r, w: eT[:, r, w:w + 512])
            nc.sync.dma_start(d_dbg_rc[:], rcol[:])

        # ------------- P4: out[s, o] = sum_t e[t,s] vo[t,o] * recip[s] ------
        for ot in range(n512):
            osl = slice(ot * 512, (ot + 1) * 512)
            for sc in range(sc_n):
                if ot == n512 - 1 and sc == sc_n - 1:
                    # Final tile as two half-width groups so the first half's
                    # ACT+DMA drains while the second half's matmuls run,
                    # shortening the end-of-program tail.
                    for hh in range(2):
                        hsl = slice(ot * 512 + hh * 256,
                                    ot * 512 + (hh + 1) * 256)
                        ps_h = pp.tile([P, 512], f32, tag="PS", name="ps_o")
                        for tch in range(tc_n):
                            nc.tensor.matmul(
                                ps_h[:, 0:256],
                                eT[:, tch, sc * P:(sc + 1) * P],
                                vo[:, tch, hsl],
                                start=(tch == 0), stop=(tch == tc_n - 1))
                        ob = pgo.tile([P, 512], f32, tag="OB", name="ob")
                        nc.scalar.activation(ob[:, 0:256], ps_h[:, 0:256],
                                             AF.Copy,
                                             scale=rcol[:, sc:sc + 1])
                        nc.sync.dma_start(d_out[sc * P:(sc + 1) * P, hsl],
                                          ob[:, 0:256])
                    continue
                ps_o = pp.tile([P, 512], f32, tag="PS", name="ps_o")
                for tch in range(tc_n):
                    nc.tensor.matmul(ps_o[:], eT[:, tch, sc * P:(sc + 1) * P],
                                     vo[:, tch, osl],
                                     start=(tch == 0), stop=(tch == tc_n - 1))
                ob = pgo.tile([P, 512], f32, tag="OB", name="ob")
                nc.scalar.activation(ob[:], ps_o[:], AF.Copy,
                                     scale=rcol[:, sc:sc + 1])
                nc.sync.dma_start(d_out[sc * P:(sc + 1) * P, osl], ob[:])

    nc.compile()
    return nc


def _to_dev_layout(x_t, rows, dtype=None):
    """[rows, n] fp32 -> [128, rows//128, n] bf16/fp8 contiguous."""
    rc = rows // P
    return np.ascontiguousarray(
        x_t.reshape(rc, P, -1).transpose(1, 0, 2).astype(dtype or bf16))


def _to_chunked_flat(x_t, rows):
    """[rows, n] fp32 -> [n//128, 128, rows] bf16, n-chunk major.

    Chunk c holds columns [c*128, (c+1)*128) as a [128(part=row%128),
    rows//128 * 128] tile whose per-partition data is contiguous (the
    [rowchunk, col] panel flattened), giving 4KB DMA runs.
    """
    dev = _to_dev_layout(x_t, rows)              # [128, rc, n]
    n = dev.shape[2]
    rc = rows // P
    # -> [n//128, 128, rc, 128] -> flatten last two
    ch = np.ascontiguousarray(
        dev.reshape(P, rc, n // P, P).transpose(2, 0, 1, 3))
    return np.ascontiguousarray(ch.reshape(n // P, P, rc * P))


def _hi_lo_f8(x):
    """Exact 2-term fp8 split: x ~= hi + lo with hi = f8(x)."""
    f8 = ml_dtypes.float8_e4m3
    hi = x.astype(f8)
    lo = (x - hi.astype(np.float32)).astype(f8)
    return hi, lo


def _to_chunked_flat_f8(x_t, rows):
    """Chunk-major like _to_chunked_flat but fp8e4."""
    f8 = ml_dtypes.float8_e4m3
    rc = rows // P
    dev = np.ascontiguousarray(
        x_t.reshape(rc, P, -1).transpose(1, 0, 2).astype(f8))
    n = dev.shape[2]
    ch = np.ascontiguousarray(
        dev.reshape(P, rc, n // P, P).transpose(2, 0, 1, 3))
    return np.ascontiguousarray(ch.reshape(n // P, P, rc * P))


def _gate_prep_merged(trip, rid, cpad):
    """Host-side gate folding for a t-slice.

    trip: [t, 3, h] fp32; rid: [t] ids.
    Returns adv_lin [t,h], sc [cpad,h], Pc [t,cpad] with impl and and/or/xor
    compact rows merged (disjoint) and one signed scatter matrix.
    """
    t_n = trip.shape[0]
    h = trip.shape[2]
    m_and = rid == 0
    m_or = rid == 1
    m_not = rid == 2
    m_impl = rid == 3
    m_xor = rid == 4
    c0 = (rid >= 5).astype(np.float32)
    ca = m_and.astype(np.float32) - m_xor.astype(np.float32)
    cb = m_or.astype(np.float32) + m_xor.astype(np.float32)
    c1 = -(m_not.astype(np.float32))
    ci = m_impl.astype(np.float32)
    k_s = (ca + cb + c1) / 2
    k_d = (c1 - ci) / 2
    k_as = ci / 2
    k_ad = (cb - ca) / 2

    a0 = trip[:, 0]
    asum = trip[:, 1] + trip[:, 2]
    adif = trip[:, 1] - trip[:, 2]
    adv_lin = c0[:, None] * a0 + k_s[:, None] * asum + k_d[:, None] * adif

    impl_idx = np.where(m_impl)[0]
    aox_idx = np.where(m_and | m_or | m_xor)[0]
    n_i, n_a = len(impl_idx), len(aox_idx)
    assert n_i + n_a <= cpad, f"compact rows {n_i + n_a} > pad {cpad}"
    sc = np.zeros((cpad, h), np.float32)
    sc[:n_i] = k_as[impl_idx, None] * asum[impl_idx]
    sc[n_i:n_i + n_a] = np.abs(k_ad[aox_idx, None]) * adif[aox_idx]
    Pc = np.zeros((t_n, cpad), np.float32)
    Pc[impl_idx, np.arange(n_i)] = 1.0
    Pc[aox_idx, n_i + np.arange(n_a)] = np.sign(k_ad[aox_idx])
    return adv_lin, sc, Pc


def kernel(hidden_states, advisor_states, advisor_ids, Wq, Wk, Wv, Wo):
    from concourse.bass_utils import run_bass_kernel_spmd
    import concourse.mybir as mybir

    hs = np.asarray(hidden_states, dtype=np.float32)     # [4, 2048, 2048]
    adv = np.asarray(advisor_states, dtype=np.float32)   # [4, 3072, 2048]
    ids = np.asarray(advisor_ids)                        # [4, 3072]

    # Compact pad per t-half, sized to the data (multiple of 128).
    rid_all = ids.reshape(B, T, 3)[:, :, 0]
    need = 0
    for b in range(B):
        for j in range(2):
            r = rid_all[b, j * TH:(j + 1) * TH]
            need = max(need, int(((r == 0) | (r == 1) | (r == 3)
                                  | (r == 4)).sum()))
    chc = max(P, -(-need // P) * P)

    global _compiled_nc
    if _compiled_nc is None or _compiled_nc[0] != chc:
        _compiled_nc = (chc, _build_nc5(chc=chc))
    nc = _compiled_nc[1]

    Wq = np.asarray(Wq, dtype=np.float32)
    Wk = np.asarray(Wk, dtype=np.float32)
    Wv = np.asarray(Wv, dtype=np.float32)
    Wo = np.asarray(Wo, dtype=np.float32)

    MT = Wk.T @ Wq                    # = (Wq.T @ Wk).T, [i, o]
    WvT = np.ascontiguousarray(Wv.T)  # [i, j]
    WoT = np.ascontiguousarray(Wo.T)  # [j, o]
    Wvo = WvT @ WoT                   # [i, o]

    # absT runs in fp8 DoubleRow: Wv.T is scaled by 16 into fp8's normal
    # range; sc is split host-side into exact hi+lo fp8 terms so only the
    # Wv-side fp8 rounding contributes error. The net 16x is folded into Pc
    # (exact +-2^-4 in bf16).
    # kM runs as a 3-term fp8 DoubleRow product: both M.T (x16 into fp8
    # range) and a0.T are split exactly into hi+lo fp8; the 16x comes back
    # out through the exp scale.
    mt_hi, mt_lo = _hi_lo_f8(np.ascontiguousarray(MT) * 16.0)
    mt_dev = np.stack(
        [_to_chunked_flat_f8(mt_hi.astype(np.float32), H),
         _to_chunked_flat_f8(mt_lo.astype(np.float32), H)], axis=1)
    w_dev = {
        "mt": np.ascontiguousarray(mt_dev),          # [hc, 2, P, h]
        "wv": _to_chunked_flat_f8(WvT * 16.0, H),
        "wo": _to_dev_layout(WoT, H),
        "wvo": _to_dev_layout(np.ascontiguousarray(Wvo), H),
    }

    in_maps = []
    for c in range(NCORES):
        b, j = c // 2, c % 2
        trip = adv[b].reshape(T, 3, H)[j * TH:(j + 1) * TH]
        rid = rid_all[b, j * TH:(j + 1) * TH]
        adv_lin, sc, Pc = _gate_prep_merged(trip, rid, chc)
        msk = np.zeros((P, 2), np.float32)
        msk[:, 1 - j] = 1.0
        f8np = ml_dtypes.float8_e4m3
        scT = np.ascontiguousarray(sc.T)              # [H, chc]
        sc_hi = scT.astype(f8np)
        sc_lo = (scT - sc_hi.astype(np.float32)).astype(f8np)
        sct_dev = np.stack([_to_dev_layout(sc_hi.astype(np.float32), H, f8np),
                            _to_dev_layout(sc_lo.astype(np.float32), H, f8np)],
                           axis=1)                    # [128, 2, hc, chc]
        a0T = np.ascontiguousarray(trip[:, 0].T)
        a0_hi, a0_lo = _hi_lo_f8(a0T)
        f8np2 = ml_dtypes.float8_e4m3
        a0t_dev = np.stack(
            [_to_dev_layout(a0_hi.astype(np.float32), H, f8np2),
             _to_dev_layout(a0_lo.astype(np.float32), H, f8np2)], axis=1)
        m = {
            "a0t": np.ascontiguousarray(a0t_dev),    # [128, 2, hc, th]
            "alt": _to_dev_layout(np.ascontiguousarray(adv_lin.T), H),
            "sct": np.ascontiguousarray(sct_dev),
            "pct": _to_dev_layout(np.ascontiguousarray(Pc.T / 16.0), chc),
            "hT": _to_dev_layout(
                np.ascontiguousarray(hs[b, j * S:(j + 1) * S, :].T), H),
            "msk": msk,
            **w_dev,
        }
        in_maps.append(m)

    res = run_bass_kernel_spmd(nc, in_maps, core_ids=list(range(NCORES)))
    kernel._last_results = res

    out = np.empty((B, 2 * S, H), dtype=np.float32)
    for c in range(NCORES):
        b, j = c // 2, c % 2
        out[b, j * S:(j + 1) * S, :] = res.results[c]["out"]
    return out


# revision 90
# speedup vs baseline: 1.0516x; 1.0516x over previous
"""AdvisorCrossAttentionAdapter Trainium2 kernel (v5).

Full inputs in, full outputs out. Sharding: 8 cores = 4 batches x 2 "halves".
Core 2b+j handles batch b; j indexes both its 1024-row query slice and its
512-triplet share of the per-batch K/V prep. The pair exchanges prep results
(2MB each way) with cheap ReduceScatter collectives; attention runs fully
local after that.

Math notes (all folds are exact; host precomputes products in fp32):
  - K-side fold: scores = hidden @ M @ a0.T with M = Wq.T @ Wk, computed as
    kM = a0 @ M.T on device (T=1024 rows, half the flops of the q-side fold
    used previously, which projected S=2048 rows).
  - Wo folded through the value path:
      vo   = adv_lin @ Wvo + Pc @ (|sc @ Wv.T| @ Wo.T),  Wvo = Wv.T @ Wo.T
      out  = softmax(scores) @ vo
    This deletes the separate ctx (attn @ v_final) + out-projection phases.
    The id-gate decomposition (adv_lin linear part, sc compact abs rows, Pc
    signed scatter) is unchanged from v2 but built per t-half.
  - Prep is row-split (by T) across the pair with LOCAL t ordering
    [own 512 | peer 512] on both kM and vo; softmax/ctx are permutation
    invariant in t so no global order is needed.
  - Pair exchange via ReduceScatter(add): each core writes its own half into
    the peer-destined shard and zeros into its own shard (two ACT copies
    scaled by a host-fed 0/1 mask keep the program SPMD-symmetric). RS output
    is half the bytes of an AllGather output, which halves the collective
    cost under the serialized COLLECTIVE_CORES device.
  - Softmax runs without max subtraction (scores/sqrt(h) ~ N(0,1)); exp'd
    scores stay unnormalized through the out matmul and the 1/sum factor is
    applied on the final ACT copy (per-partition scale).
  - All matmuls take bf16 inputs with fp32 PSUM accumulation.
"""

import numpy as np
import ml_dtypes
from contextlib import ExitStack

P = 128
H = 2048          # hidden dim
HC = H // P       # 16 h-chunks of 128
T = 1024          # triplets per batch (advisor len 3072 / 3)
TC = T // P       # 8 t-chunks
TH = T // 2       # own t-half rows (512)
THC = TH // P     # 4 own t-chunks
S = 1024          # query rows per core (2048 / 2)
B = 4
NCORES = 8
CHC_MIN = 256     # compact rows per t-half, padded to 128 (data-keyed)
SCALE = 1.0 / float(np.sqrt(H))

bf16 = ml_dtypes.bfloat16

_compiled_nc = None


def _build_nc5(s_rows=S, t_trip=T, h=H, chc=CHC_MIN, n_dev=NCORES,
               debug=False):
    import concourse.mybir as mybir
    import concourse.tile as tile
    from concourse import bacc

    hc = h // P
    tc_n = t_trip // P
    th = t_trip // 2
    thc = th // P
    sc_n = s_rows // P
    s512 = s_rows // 512
    n512 = h // 512
    chb = chc // P
    assert s_rows % 512 == 0 and h % 1024 == 0 and chc % P == 0

    f32 = mybir.dt.float32
    bf = mybir.dt.bfloat16
    f8 = mybir.dt.float8e4

    nc = bacc.Bacc("TRN2", target_bir_lowering=False, debug=False,
                   num_devices=n_dev)

    # DRAM I/O (all bf16, host pre-transposed):
    #   mt  [hc, P, h]  : chunked M.T  (o-chunk major; lhsT tiles [i, o])
    #   a0t [P, hc, th] : a0.T own t-half columns (rhs for kM)
    #   wv  [hc, P, h]  : chunked Wv.T (j-chunk major; lhsT tiles [i, j])
    #   sct [P, hc, chc]: sc.T own compact rows (rhs for absT)
    #   wo  [P, hc, h]  : Wo.T  (rhs stream for acWo)
    #   wvo [P, hc, h]  : (Wv.T @ Wo.T)  (rhs stream for vo_lin)
    #   alt [P, hc, th] : adv_lin.T own columns (lhsT for vo_lin)
    #   pct [P, chb, th]: Pc.T own (lhsT for scatter)
    #   hT  [P, hc, s]  : hidden.T own query slice (rhs for scores)
    #   msk [P, 2] f32  : RS shard masks (1.0 on peer-destined shard)
    d_mt = nc.dram_tensor("mt", [hc, 2, P, h], f8, kind="ExternalInput")
    d_a0t = nc.dram_tensor("a0t", [P, 2, hc, th], f8, kind="ExternalInput")
    d_wv = nc.dram_tensor("wv", [hc, P, h], f8, kind="ExternalInput")
    d_sct = nc.dram_tensor("sct", [P, 2, hc, chc], f8, kind="ExternalInput")
    d_wo = nc.dram_tensor("wo", [P, hc, h], bf, kind="ExternalInput")
    d_wvo = nc.dram_tensor("wvo", [2, P, hc, h], f8, kind="ExternalInput")
    d_alt = nc.dram_tensor("alt", [P, 2, hc, th], f8, kind="ExternalInput")
    d_pct = nc.dram_tensor("pct", [P, chb, th], bf, kind="ExternalInput")
    d_h = nc.dram_tensor("hT", [P, hc, s_rows], bf, kind="ExternalInput")
    d_msk = nc.dram_tensor("msk", [P, 2], f32, kind="ExternalInput")
    d_out = nc.dram_tensor("out", [s_rows, h], f32, kind="ExternalOutput")
    if debug:
        d_dbg_km = nc.dram_tensor("dbg_km", [h, t_trip], f32,
                                  kind="ExternalOutput")
        d_dbg_vo = nc.dram_tensor("dbg_vo", [t_trip, h], f32,
                                  kind="ExternalOutput")
        d_dbg_e = nc.dram_tensor("dbg_e", [t_trip, s_rows], f32,
                                 kind="ExternalOutput")
        d_dbg_rc = nc.dram_tensor("dbg_rc", [P, 8], f32,
                                  kind="ExternalOutput")

    AF = mybir.ActivationFunctionType
    pairs = [[2 * i, 2 * i + 1] for i in range(n_dev // 2)]

    with tile.TileContext(nc) as tc, ExitStack() as ctx:
        big = ctx.enter_context(tc.tile_pool(name="big", bufs=1))
        pws = ctx.enter_context(tc.tile_pool(name="pws", bufs=6))
        pw = ctx.enter_context(tc.tile_pool(name="pw", bufs=3))
        pgs = ctx.enter_context(tc.tile_pool(name="pgs", bufs=4))
        pgo = ctx.enter_context(tc.tile_pool(name="pgo", bufs=3))
        psm = ctx.enter_context(tc.tile_pool(name="psm", bufs=1))
        pp = ctx.enter_context(tc.tile_pool(name="pp", bufs=8, space="PSUM"))
        dram = ctx.enter_context(tc.tile_pool(name="dram", bufs=1,
                                              space="DRAM"))

        # Persistent SBUF intermediates. Tag chains reuse slots across phases:
        #   tag A: a0t (P1)  -> hT (P3)      [32KB/part]
        #   tag L: alt (P2)  -> eT (P3/P4)   [16KB/part]
        kmT = big.tile([P, hc, t_trip], bf, tag="K", name="kmT")
        vo = big.tile([P, tc_n, h], bf, tag="V", name="vo")
        a0t_sb = big.tile([P, 2, hc, th], f8, tag="A", name="a0t_sb")
        alt_sb = big.tile([P, 2, hc, th], f8, tag="L", name="alt_sb")
        sct_sb = big.tile([P, 2, hc, chc], f8, tag="S", name="sct_sb")
        absT_sb = big.tile([P, hc, chc], bf, tag="B", name="absT_sb")
        acWo_sb = big.tile([P, chb, h], bf, tag="W", name="acWo_sb")
        pct_sb = psm.tile([P, chb, th], bf, tag="pc", name="pct_sb")
        msk_sb = psm.tile([P, 2], f32, tag="mk", name="msk_sb")

        # Exchange buffers (internal DRAM). km_in shard s holds this core's
        # kM.T half scaled by msk[s]; RS(add) then delivers the peer's half.
        km_in = dram.tile([2, h, th], bf, name="km_in", uniquify=False)
        km_out = dram.tile([h, th], bf, name="km_out", uniquify=False)
        # Permuted views so ONE staging DMA per PSUM tile writes both shards.
        km_rv = km_in.rearrange("s (oc p) t -> p oc s t", p=P)
        vo_in = []
        vo_out = []
        vo_rv = []
        for k in range(2):
            vo_in.append(dram.tile([2, th, h // 2], bf, name=f"vo_in{k}",
                                   uniquify=False))
            vo_out.append(dram.tile([th, h // 2], bf, name=f"vo_out{k}",
                                    uniquify=False))
            vo_rv.append(vo_in[k].rearrange("s (tb p) o -> p tb s o", p=P))

        # First critical input DMAs. a0t quarter-split (alternating queues)
        # so P1's first matmul group can start before the whole 2MB lands.
        # Queue discipline: sync carries ONLY pure input streams (never a DMA
        # that waits on local compute), so it can never head-of-line block;
        # compute-dependent staging writes all go on scalar.
        qs = max(1, hc // 4)
        nc.scalar.dma_start(a0t_sb[:, 0, 0:hc // 2, :],
                            d_a0t[:, 0, 0:hc // 2, :])
        nc.scalar.dma_start(a0t_sb[:, 0, hc // 2:hc, :],
                            d_a0t[:, 0, hc // 2:hc, :])
        nc.gpsimd.dma_start(msk_sb[:], d_msk[:])
        nc.gpsimd.dma_start(pct_sb[:], d_pct[:])

        # ACT-written zero bias vector (x*0) so Abs/Exp activations don't pull
        # in a DMA'd const AP; also absorbs the pipeline-RAW wait.
        zbias = psm.tile([P, 1], f32, tag="zb", name="zbias")
        nc.scalar.mul(zbias[:], msk_sb[:, 0:1], 0.0)
        warm = psm.tile([P, 1], f32, tag="wm", name="warm")
        nc.scalar.copy(warm[:], zbias[:])

        # PE warm-up: throwaway matmuls while the first weight tiles land so
        # the PE p-state ramp completes before real groups start.
        dummy = psm.tile([P, 512], bf, tag="dm", name="dummy")
        nc.vector.memset(dummy[:], 0.0)
        for _ in range(16):
            ps_dm = pp.tile([P, 512], f32, tag="PS", name="ps_dm")
            nc.tensor.matmul(ps_dm[:], dummy[:, 0:P], dummy[:],
                             start=True, stop=True)

        # ------------- P1: kM.T own columns = M @ a0_own.T ------------------
        # kmT local t order: [own 512 | peer 512]. The mt/wv streams alternate
        # sync/scalar and are issued AHEAD of the compute-dependent staging
        # writes in FIFO order (wv prefetches are hoisted into the P1 loop) so
        # the queues never head-of-line block the weight streams.
        npre = 5
        stream_tiles = {}

        def q_of(i):
            # Streams ride the scalar (ACT) HWDGE queue; compute-dependent
            # writes ride sync (SP) so their desc-gen never clogs ACT's SEQ
            # and streams never sit behind compute-gated writes.
            return nc.scalar

        for oc in range(npre):
            mt_oc = pws.tile([P, 2, hc, P], f8, tag="W1", name="mt_oc")
            stream_tiles[oc] = mt_oc
            if oc == 0:
                nc.scalar.dma_start(mt_oc[:, 0], d_mt[oc, 0])
                nc.scalar.dma_start(mt_oc[:, 1], d_mt[oc, 1])
                # a0t lo-term lands while group 0's hi terms run.
                nc.scalar.dma_start(a0t_sb[:, 1, 0:hc // 2, :],
                                    d_a0t[:, 1, 0:hc // 2, :])
                nc.scalar.dma_start(a0t_sb[:, 1, hc // 2:hc, :],
                                    d_a0t[:, 1, hc // 2:hc, :])
            else:
                q_of(oc).dma_start(mt_oc[:, 0], d_mt[oc, 0])
                q_of(oc + 1).dma_start(mt_oc[:, 1], d_mt[oc, 1])
        wtiles = {}
        for oc in range(hc):
            nx = oc + npre
            if nx < hc:
                t_nx = pws.tile([P, 2, hc, P], f8, tag="W1", name="mt_oc")
                stream_tiles[nx] = t_nx
                q_of(nx).dma_start(t_nx[:, 0], d_mt[nx, 0])
                q_of(nx + 1).dma_start(t_nx[:, 1], d_mt[nx, 1])
            else:
                # wv (fp8) comes in PAIRS: two 2KB tiles share one 4KB slot,
                # doubling the prefetch depth for the DMA-bound absT phase.
                pq2 = nx - hc
                t_nx = pws.tile([P, 2, hc, P], f8, tag="W1", name="wv_jc")
                stream_tiles[hc + pq2] = t_nx
                q_of(nx).dma_start(t_nx[:, 0], d_wv[2 * pq2])
                q_of(nx + 1).dma_start(t_nx[:, 1], d_wv[2 * pq2 + 1])
            if oc == 6:
                # sct (hi/lo fp8 terms) lands mid-P1 (needed from P2 on).
                nc.sync.dma_start(sct_sb[:, 0, :, :], d_sct[:, 0, :, :])
                nc.scalar.dma_start(sct_sb[:, 1, :, :], d_sct[:, 1, :, :])
            if oc in (12, 14):
                # Prefetch the first two wo tiles during P1's tail.
                wi = (oc - 12) // 2
                wt = pw.tile([P, hc, 256], bf, tag="W4", name="wo_ot")
                wtiles[("wo", wi)] = wt
                q_of(oc).dma_start(wt[:],
                                   d_wo[:, :, wi * 256:(wi + 1) * 256])
            mt_oc = stream_tiles.pop(oc)
            # 16x-scaled kM via 3-term fp8 DoubleRow (hi.hi + hi.lo + lo.hi;
            # both splits are exact host-side, so accuracy ~bf16). Two
            # 256-wide halves in SEPARATE banks (start resets a whole bank).
            pshs = []
            for hf in range(2):
                hsl = slice(hf * 256, (hf + 1) * 256)
                ps_k = pp.tile([P, 512], f32, tag="PS", name="ps_k")
                pshs.append(ps_k)
                for tn, (ta, tb2) in enumerate(((0, 0), (0, 1), (1, 0))):
                    for ii in range(hc // 2):
                        nc.tensor.matmul(
                            ps_k[:, 0:256],
                            mt_oc[:, ta, 2 * ii:2 * ii + 2, :],
                            a0t_sb[:, tb2, 2 * ii:2 * ii + 2, hsl],
                            start=(tn == 0 and ii == 0),
                            stop=(tn == 2 and ii == hc // 2 - 1),
                            perf_mode=mybir.MatmulPerfMode.DoubleRow)
                nc.vector.tensor_copy(kmT[:, oc, hsl], ps_k[:, 0:256])
            stg = pgs.tile([P, 2, th], bf, tag="VS", name="stg_k")
            for sh in range(2):
                for hf in range(2):
                    nc.scalar.activation(
                        stg[:, sh, hf * 256:(hf + 1) * 256],
                        pshs[hf][:, 0:256], AF.Copy,
                        scale=msk_sb[:, sh:sh + 1])
            nc.sync.dma_start(km_rv[:, oc], stg[:])
        nc.gpsimd.collective_compute(
            "ReduceScatter",
            mybir.AluOpType.add,
            replica_groups=pairs,
            ins=[km_in.opt()],
            outs=[km_out.opt()],
        )
        # The kM scatter-back is emitted AFTER the first vo RS (see below):
        # emitted here it would hold the gpsimd SEQ through its wait on the
        # kM RS, delaying the vo RS dispatches by ~20us.

        # ------------- P2: value path -> vo (local t order) -----------------
        # absT[j, c] = |sc @ Wv.T|.T for own compact rows.
        npair = hc // 2
        for q in range(npair):
            nxp = q + npre
            if nxp < npair:
                t_nx = pws.tile([P, 2, hc, P], f8, tag="W1", name="wv_jc")
                stream_tiles[hc + nxp] = t_nx
                q_of(nxp).dma_start(t_nx[:, 0], d_wv[2 * nxp])
                q_of(nxp + 1).dma_start(t_nx[:, 1], d_wv[2 * nxp + 1])
            wv_pair = stream_tiles.pop(hc + q)
            for half in range(2):
                jc = 2 * q + half
                ps_a = pp.tile([P, 512], f32, tag="PS", name="ps_a")
                for term in range(2):
                    for ii in range(hc // 2):
                        nc.tensor.matmul(
                            ps_a[:, 0:chc],
                            wv_pair[:, half, 2 * ii:2 * ii + 2, :],
                            sct_sb[:, term, 2 * ii:2 * ii + 2, :],
                            start=(term == 0 and ii == 0),
                            stop=(term == 1 and ii == hc // 2 - 1),
                            perf_mode=mybir.MatmulPerfMode.DoubleRow)
                nc.scalar.activation(absT_sb[:, jc, :], ps_a[:, 0:chc],
                                     AF.Abs, bias=zbias[:])

        # hT for P3 (reuses a0t's slot via tag A chain; a0t is dead after
        # P1). Loaded in 512KB pieces spread through the vo loop and P3-st0
        # so no single transfer hogs the serial DMA pool during P2's crunch.
        hT = big.tile([P, hc, s_rows], bf, tag="A", name="hT")

        # acWo[c, o] = absT.T @ Wo.T over all o, per half-o-tile (256 cols).
        # The wo/wvo streams keep a 2-deep lookahead on alternating queues;
        # alt and hT-half0 pieces ride this window's DMA slack.
        ho = 256
        nho = h // ho
        per_k = nho // 2
        issued = set(wtiles.keys())

        def fetch_w(key):
            if key in issued:
                return
            issued.add(key)
            kind, i = key
            wt = pw.tile([P, hc, ho], bf, tag="W4", name=f"{kind}_ot")
            wtiles[key] = wt
            q_of(i).dma_start(wt[:], d_wo[:, :, i * ho:(i + 1) * ho])

        # wvo (x16, fp8 hi/lo) streams in 512-wide per-term tiles through a
        # hand-rolled 4-slot rotation: sct's dead 8KB big-slot + pw's three
        # slots (dead after the wo stream). hi_k/lo_k live in
        # wv4[(2k)%4]/wv4[(2k+1)%4]; fetches are staggered one half-o-tile
        # ahead so every 512-block is fully double-buffered.
        wv4 = [big.tile([P, hc, 512], f8, tag="S", name="wvo_s")]
        wv4_ready = [False] * 8

        def fetch_wvo(idx):
            # idx = 2k (hi of block k) or 2k+1 (lo of block k); 5-slot
            # rotation (sct + 3 pw + absT dead slots) so each re-fetch waits
            # a read ~2.5 blocks back.
            if idx >= 8 or wv4_ready[idx]:
                return
            wv4_ready[idx] = True
            while len(wv4) < min(5, idx + 1):
                if len(wv4) < 4:
                    wv4.append(pw.tile([P, hc, 512], f8, tag="W4",
                                       name="wvo_p"))
                else:
                    wv4.append(big.tile([P, hc, 512], f8, tag="B",
                                        name="wvo_b"))
            term, blk = idx % 2, idx // 2
            q_of(idx).dma_start(wv4[idx % 5][:],
                                d_wvo[term, :, :, blk * 512:(blk + 1) * 512])

        for ot in range(nho):
            osl = slice(ot * ho, (ot + 1) * ho)
            for nx in (ot + 1, ot + 2):
                if nx < nho:
                    fetch_w(("wo", nx))
            if ot < 2:
                q_of(ot).dma_start(alt_sb[:, ot, :, :], d_alt[:, ot, :, :])
            if ot >= nho - 3:
                # hi0 (big-S slot, sct long dead), then lo0/hi1 as the wo
                # slots free up.
                fetch_wvo(ot - (nho - 3))
            if ot >= nho - 4:
                # First s-half of hT, in 512KB pieces, on sync (idle here).
                pq = (ot - (nho - 4)) * qs
                nc.sync.dma_start(hT[:, pq:pq + qs, 0:512],
                                  d_h[:, pq:pq + qs, 0:512])
            wo_ot = wtiles.pop(("wo", ot))
            for cb in range(chb):
                ps_c = pp.tile([P, 512], f32, tag="PS", name="ps_c")
                for jc in range(hc):
                    nc.tensor.matmul(ps_c[:, 0:ho],
                                     absT_sb[:, jc, cb * P:(cb + 1) * P],
                                     wo_ot[:, jc, :],
                                     start=(jc == 0), stop=(jc == hc - 1))
                nc.vector.tensor_copy(acWo_sb[:, cb, osl], ps_c[:, 0:ho])
        # vo own rows = 16*(adv_lin @ Wvo + Pc' @ acWo) via 3-term fp8
        # DoubleRow (both operands exactly hi/lo split); the 1/16 comes back
        # through the ones-column row sums (ones_t = 16).
        fetch_wvo(3)
        for ot in range(nho):
            osl = slice(ot * ho, (ot + 1) * ho)
            fetch_wvo(ot + 3)
            blk, wsl = ot // 2, slice((ot % 2) * ho, (ot % 2 + 1) * ho)
            w_hi = wv4[(2 * blk) % 5]
            w_lo = wv4[(2 * blk + 1) % 5]
            k, col = ot // per_k, (ot % per_k) * ho
            for tb in range(thc):
                ps_v = pp.tile([P, 512], f32, tag="PS", name="ps_v")
                for tn, (ta, wt_) in enumerate(
                        ((0, w_hi), (1, w_hi), (0, w_lo))):
                    for ii in range(hc // 2):
                        nc.tensor.matmul(
                            ps_v[:, 0:ho],
                            alt_sb[:, ta, 2 * ii:2 * ii + 2,
                                   tb * P:(tb + 1) * P],
                            wt_[:, 2 * ii:2 * ii + 2, wsl],
                            start=(tn == 0 and ii == 0), stop=False,
                            perf_mode=mybir.MatmulPerfMode.DoubleRow)
                for cb in range(chb):
                    nc.tensor.matmul(ps_v[:, 0:ho],
                                     pct_sb[:, cb, tb * P:(tb + 1) * P],
                                     acWo_sb[:, cb, osl],
                                     start=False, stop=(cb == chb - 1))
                nc.vector.tensor_copy(vo[:, tb, osl], ps_v[:, 0:ho])
                stg = pgs.tile([P, 2, ho], bf, tag="VS2", name="stg_v")
                for sh in range(2):
                    nc.scalar.activation(stg[:, sh, :], ps_v[:, 0:ho],
                                         AF.Copy,
                                         scale=msk_sb[:, sh:sh + 1])
                nc.sync.dma_start(vo_rv[k][:, tb, :, col:col + ho], stg[:])
            if (ot + 1) % per_k == 0:
                nc.gpsimd.collective_compute(
                    "ReduceScatter",
                    mybir.AluOpType.add,
                    replica_groups=pairs,
                    ins=[vo_in[k].opt()],
                    outs=[vo_out[k].opt()],
                )
                if k == 0:
                    # Peer kM columns land via SWDGE. Emitted here (after the
                    # vo chunk-0 RS dispatch) its RS1-sem wait is already
                    # satisfied, so it never blocks the gpsimd SEQ.
                    nc.gpsimd.dma_start(
                        kmT[:, :, th:t_trip],
                        km_out.rearrange("(oc p) t -> p oc t", p=P))
                nc.gpsimd.dma_start(
                    vo[:, thc:tc_n, k * (h // 2):(k + 1) * (h // 2)],
                    vo_out[k].rearrange("(tb p) o -> p tb o", p=P))

        # ------------- P3: scores^T, exp, sums ------------------------------
        # st-major so the first s-half only needs the first half of hT. Row
        # sums accumulate directly into per-s-chunk PSUM columns ([128,1]
        # matmuls against a ones column), giving per-partition reciprocals
        # with no transpose step; emission is deferred one t-chunk so the PE
        # never waits on the ACT exp.
        eT = big.tile([P, tc_n, s_rows], bf, tag="L", name="eT")
        ones_t = psm.tile([P, 1], bf, tag="o1", name="ones_t")
        # 16.0: vo carries a 16x scale (fp8-range Wvo); folding 1/16 into the
        # row sums makes rcol = 1/(16*sum), normalizing it away exactly.
        nc.vector.memset(ones_t[:], 16.0)
        rcol = psm.tile([P, sc_n], f32, tag="rl", name="rcol")

        def emit_sums(st):
            # Row sums as [128,1] accumulation groups, one PSUM TILE each
            # (start resets the whole addressed bank row per partition, so
            # distinct groups must use distinct banks). Output is already
            # [s-part, 1]: the reciprocal lands straight in rcol.
            for sc2 in range(4):
                sc = st * 4 + sc2
                ps_s = pp.tile([P, 512], f32, tag="PS", name="ps_s")
                for tch in range(tc_n):
                    nc.tensor.matmul(ps_s[:, 0:1],
                                     eT[:, tch, sc * P:(sc + 1) * P],
                                     ones_t[:],
                                     start=(tch == 0), stop=(tch == tc_n - 1))
                nc.vector.reciprocal(rcol[:, sc:sc + 1], ps_s[:, 0:1])

        for st in range(s512):
            for tch in range(tc_n):
                if st == 0 and tch >= tc_n - 4:
                    # Second s-half of hT lands piecewise during st=0.
                    pq = (tch - (tc_n - 4)) * qs
                    q_of(tch).dma_start(hT[:, pq:pq + qs, 512:s_rows],
                                        d_h[:, pq:pq + qs, 512:s_rows])
                ps_x = pp.tile([P, 512], f32, tag="PS", name="ps_sc")
                for oc in range(hc):
                    nc.tensor.matmul(ps_x[:],
                                     kmT[:, oc, tch * P:(tch + 1) * P],
                                     hT[:, oc, st * 512:(st + 1) * 512],
                                     start=(oc == 0), stop=(oc == hc - 1))
                if st > 0 and tch == 1:
                    emit_sums(st - 1)
                nc.scalar.activation(eT[:, tch, st * 512:(st + 1) * 512],
                                     ps_x[:], AF.Exp, bias=zbias[:],
                                     scale=SCALE / 16.0)
            if st == s512 - 1:
                emit_sums(st)
        warm2 = psm.tile([P, 1], f32, tag="w2", name="warm2")
        nc.scalar.copy(warm2[:], rcol[:, 0:1])

        if debug:
            def dump(dst, rows, width, src_of):
                for r in range(rows):
                    for w in range(0, width, 512):
                        dd = pgo.tile([P, 512], f32, tag="OB", name="ob")
                        nc.vector.tensor_copy(dd[:], src_of(r, w))
                        nc.sync.dma_start(
                            dst[r * P:(r + 1) * P, w:w + 512], dd[:])
            dump(d_dbg_km, hc, t_trip, lambda r, w: kmT[:, r, w:w + 512])
            dump(d_dbg_vo, tc_n, h, lambda r, w: vo[:, r, w:w + 512])
            dump(d_dbg_e, tc_n, s_rows, lambda r, w: eT[:, r, w:w + 512])
            nc.sync.dma_start(d_dbg_rc[:], rcol[:])

        # ------------- P4: out[s, o] = sum_t e[t,s] vo[t,o] * recip[s] ------
        for ot in range(n512):
            osl = slice(ot * 512, (ot + 1) * 512)
            for sc in range(sc_n):
                if ot == n512 - 1 and sc == sc_n - 1:
                    # Final tile as two half-width groups so the first half's
                    # ACT+DMA drains while the second half's matmuls run,
                    # shortening the end-of-program tail.
                    for hh in range(2):
                        hsl = slice(ot * 512 + hh * 256,
                                    ot * 512 + (hh + 1) * 256)
                        ps_h = pp.tile([P, 512], f32, tag="PS", name="ps_o")
                        for tch in range(tc_n):
                            nc.tensor.matmul(
                                ps_h[:, 0:256],
                                eT[:, tch, sc * P:(sc + 1) * P],
                                vo[:, tch, hsl],
                                start=(tch == 0), stop=(tch == tc_n - 1))
                        ob = pgo.tile([P, 512], f32, tag="OB", name="ob")
                        nc.scalar.activation(ob[:, 0:256], ps_h[:, 0:256],
                                             AF.Copy,
                                             scale=rcol[:, sc:sc + 1])
                        nc.sync.dma_start(d_out[sc * P:(sc + 1) * P, hsl],
                                          ob[:, 0:256])
                    continue
                ps_o = pp.tile([P, 512], f32, tag="PS", name="ps_o")
                for tch in range(tc_n):
                    nc.tensor.matmul(ps_o[:], eT[:, tch, sc * P:(sc + 1) * P],
                                     vo[:, tch, osl],
                                     start=(tch == 0), stop=(tch == tc_n - 1))
                ob = pgo.tile([P, 512], f32, tag="OB", name="ob")
                nc.scalar.activation(ob[:], ps_o[:], AF.Copy,
                                     scale=rcol[:, sc:sc + 1])
                nc.sync.dma_start(d_out[sc * P:(sc + 1) * P, osl], ob[:])

    nc.compile()
    return nc


def _to_dev_layout(x_t, rows, dtype=None):
    """[rows, n] fp32 -> [128, rows//128, n] bf16/fp8 contiguous."""
    rc = rows // P
    return np.ascontiguousarray(
        x_t.reshape(rc, P, -1).transpose(1, 0, 2).astype(dtype or bf16))


def _to_chunked_flat(x_t, rows):
    """[rows, n] fp32 -> [n//128, 128, rows] bf16, n-chunk major.

    Chunk c holds columns [c*128, (c+1)*128) as a [128(part=row%128),
    rows//128 * 128] tile whose per-partition data is contiguous (the
    [rowchunk, col] panel flattened), giving 4KB DMA runs.
    """
    dev = _to_dev_layout(x_t, rows)              # [128, rc, n]
    n = dev.shape[2]
    rc = rows // P
    # -> [n//128, 128, rc, 128] -> flatten last two
    ch = np.ascontiguousarray(
        dev.reshape(P, rc, n // P, P).transpose(2, 0, 1, 3))
    return np.ascontiguousarray(ch.reshape(n // P, P, rc * P))


def _hi_lo_f8(x):
    """Exact 2-term fp8 split: x ~= hi + lo with hi = f8(x)."""
    f8 = ml_dtypes.float8_e4m3
    hi = x.astype(f8)
    lo = (x - hi.astype(np.float32)).astype(f8)
    return hi, lo


def _to_chunked_flat_f8(x_t, rows):
    """Chunk-major like _to_chunked_flat but fp8e4."""
    f8 = ml_dtypes.float8_e4m3
    rc = rows // P
    dev = np.ascontiguousarray(
        x_t.reshape(rc, P, -1).transpose(1, 0, 2).astype(f8))
    n = dev.shape[2]
    ch = np.ascontiguousarray(
        dev.reshape(P, rc, n // P, P).transpose(2, 0, 1, 3))
    return np.ascontiguousarray(ch.reshape(n // P, P, rc * P))


def _gate_prep_merged(trip, rid, cpad):
    """Host-side gate folding for a t-slice.

    trip: [t, 3, h] fp32; rid: [t] ids.
    Returns adv_lin [t,h], sc [cpad,h], Pc [t,cpad] with impl and and/or/xor
    compact rows merged (disjoint) and one signed scatter matrix.
    """
    t_n = trip.shape[0]
    h = trip.shape[2]
    m_and = rid == 0
    m_or = rid == 1
    m_not = rid == 2
    m_impl = rid == 3
    m_xor = rid == 4
    c0 = (rid >= 5).astype(np.float32)
    ca = m_and.astype(np.float32) - m_xor.astype(np.float32)
    cb = m_or.astype(np.float32) + m_xor.astype(np.float32)
    c1 = -(m_not.astype(np.float32))
    ci = m_impl.astype(np.float32)
    k_s = (ca + cb + c1) / 2
    k_d = (c1 - ci) / 2
    k_as = ci / 2
    k_ad = (cb - ca) / 2

    a0 = trip[:, 0]
    asum = trip[:, 1] + trip[:, 2]
    adif = trip[:, 1] - trip[:, 2]
    adv_lin = c0[:, None] * a0 + k_s[:, None] * asum + k_d[:, None] * adif

    impl_idx = np.where(m_impl)[0]
    aox_idx = np.where(m_and | m_or | m_xor)[0]
    n_i, n_a = len(impl_idx), len(aox_idx)
    assert n_i + n_a <= cpad, f"compact rows {n_i + n_a} > pad {cpad}"
    sc = np.zeros((cpad, h), np.float32)
    sc[:n_i] = k_as[impl_idx, None] * asum[impl_idx]
    sc[n_i:n_i + n_a] = np.abs(k_ad[aox_idx, None]) * adif[aox_idx]
    Pc = np.zeros((t_n, cpad), np.float32)
    Pc[impl_idx, np.arange(n_i)] = 1.0
    Pc[aox_idx, n_i + np.arange(n_a)] = np.sign(k_ad[aox_idx])
    return adv_lin, sc, Pc


def kernel(hidden_states, advisor_states, advisor_ids, Wq, Wk, Wv, Wo):
    from concourse.bass_utils import run_bass_kernel_spmd
    import concourse.mybir as mybir

    hs = np.asarray(hidden_states, dtype=np.float32)     # [4, 2048, 2048]
    adv = np.asarray(advisor_states, dtype=np.float32)   # [4, 3072, 2048]
    ids = np.asarray(advisor_ids)                        # [4, 3072]

    # Compact pad per t-half, sized to the data (multiple of 128).
    rid_all = ids.reshape(B, T, 3)[:, :, 0]
    need = 0
    for b in range(B):
        for j in range(2):
            r = rid_all[b, j * TH:(j + 1) * TH]
            need = max(need, int(((r == 0) | (r == 1) | (r == 3)
                                  | (r == 4)).sum()))
    chc = max(P, -(-need // P) * P)

    global _compiled_nc
    if _compiled_nc is None or _compiled_nc[0] != chc:
        _compiled_nc = (chc, _build_nc5(chc=chc))
    nc = _compiled_nc[1]

    Wq = np.asarray(Wq, dtype=np.float32)
    Wk = np.asarray(Wk, dtype=np.float32)
    Wv = np.asarray(Wv, dtype=np.float32)
    Wo = np.asarray(Wo, dtype=np.float32)

    MT = Wk.T @ Wq                    # = (Wq.T @ Wk).T, [i, o]
    WvT = np.ascontiguousarray(Wv.T)  # [i, j]
    WoT = np.ascontiguousarray(Wo.T)  # [j, o]
    Wvo = WvT @ WoT                   # [i, o]

    # absT runs in fp8 DoubleRow: Wv.T is scaled by 16 into fp8's normal
    # range; sc is split host-side into exact hi+lo fp8 terms so only the
    # Wv-side fp8 rounding contributes error. The net 16x is folded into Pc
    # (exact +-2^-4 in bf16).
    # kM runs as a 3-term fp8 DoubleRow product: both M.T (x16 into fp8
    # range) and a0.T are split exactly into hi+lo fp8; the 16x comes back
    # out through the exp scale.
    mt_hi, mt_lo = _hi_lo_f8(np.ascontiguousarray(MT) * 16.0)
    mt_dev = np.stack(
        [_to_chunked_flat_f8(mt_hi.astype(np.float32), H),
         _to_chunked_flat_f8(mt_lo.astype(np.float32), H)], axis=1)
    wvo_hi, wvo_lo = _hi_lo_f8(np.ascontiguousarray(Wvo) * 16.0)
    f8np0 = ml_dtypes.float8_e4m3
    wvo_dev = np.stack(
        [_to_dev_layout(wvo_hi.astype(np.float32), H, f8np0),
         _to_dev_layout(wvo_lo.astype(np.float32), H, f8np0)], axis=0)
    w_dev = {
        "mt": np.ascontiguousarray(mt_dev),          # [hc, 2, P, h]
        "wv": _to_chunked_flat_f8(WvT * 16.0, H),
        "wo": _to_dev_layout(WoT, H),
        "wvo": np.ascontiguousarray(wvo_dev),        # [2, 128, hc, h]
    }

    in_maps = []
    for c in range(NCORES):
        b, j = c // 2, c % 2
        trip = adv[b].reshape(T, 3, H)[j * TH:(j + 1) * TH]
        rid = rid_all[b, j * TH:(j + 1) * TH]
        adv_lin, sc, Pc = _gate_prep_merged(trip, rid, chc)
        msk = np.zeros((P, 2), np.float32)
        msk[:, 1 - j] = 1.0
        f8np = ml_dtypes.float8_e4m3
        scT = np.ascontiguousarray(sc.T)              # [H, chc]
        sc_hi = scT.astype(f8np)
        sc_lo = (scT - sc_hi.astype(np.float32)).astype(f8np)
        sct_dev = np.stack([_to_dev_layout(sc_hi.astype(np.float32), H, f8np),
                            _to_dev_layout(sc_lo.astype(np.float32), H, f8np)],
                           axis=1)                    # [128, 2, hc, chc]
        a0T = np.ascontiguousarray(trip[:, 0].T)
        a0_hi, a0_lo = _hi_lo_f8(a0T)
        f8np2 = ml_dtypes.float8_e4m3
        a0t_dev = np.stack(
            [_to_dev_layout(a0_hi.astype(np.float32), H, f8np2),
             _to_dev_layout(a0_lo.astype(np.float32), H, f8np2)], axis=1)
        m = {
            "a0t": np.ascontiguousarray(a0t_dev),    # [128, 2, hc, th]
            "alt": np.ascontiguousarray(np.stack(
                [_to_dev_layout(a.astype(np.float32), H, f8np2)
                 for a in _hi_lo_f8(np.ascontiguousarray(adv_lin.T))],
                axis=1)),
            "sct": np.ascontiguousarray(sct_dev),
            "pct": _to_dev_layout(np.ascontiguousarray(Pc.T), chc),
            "hT": _to_dev_layout(
                np.ascontiguousarray(hs[b, j * S:(j + 1) * S, :].T), H),
            "msk": msk,
            **w_dev,
        }
        in_maps.append(m)

    res = run_bass_kernel_spmd(nc, in_maps, core_ids=list(range(NCORES)))
    kernel._last_results = res

    out = np.empty((B, 2 * S, H), dtype=np.float32)
    for c in range(NCORES):
        b, j = c // 2, c % 2
        out[b, j * S:(j + 1) * S, :] = res.results[c]["out"]
    return out
